# revision 3
# baseline (speedup 1.0000x reference)
"""BitDelta linear on 8 TRN2 NeuronCores — v2: pure streaming GEMM.

C[b,s,o] = sum_i X[b,s,i] * (base[o,i] + (2*signs[o,i]-1)*scales[o])

Sharding: 2 token-groups x 4 feature-groups (tensor-parallel on out_features
per the hint + token split). Per core: X_c [8192, 4096] @ W_c^T [1024, 4096]
-> out_c [8192, 1024] f32.

v2 key change vs baseline: ALL data marshalling is done host-side (the
baseline already repacked dtypes host-side; v2 also pre-applies the delta
and pre-transposes both operands to k-major tiled layouts).  The device
kernel is a pure LDW+MM stream:

  xt [TT=64, 128, 4096] bf16 : xt[t, p, kt*128+tok] = X_c[t*128+tok, kt*128+p]
  wt [128, KT=32, 1024] bf16 : wt[p, kt, f]         = W_c[f, kt*128+p]

  for t in 64:  (xt tile DMA, triple buffered)
    for f in 2:  # f-outer: bank f's evac overlaps bank f+1's chain
      for k in 32: matmul(ps[f], xt[:,k,:], wt[:,k,f*512:], start/stop)
    evac both banks (DVE, f32->bf16) -> out DMA (bf16, host casts back)

PE work is exactly the 4096 N=512 matmuls = 874 us/core floor; no
id-matmul transposes, no on-device weight prep.  Weights double-buffer
across repeat iterations so the 8 MB W stream overlaps the previous
iteration's tail compute.
"""

import sys

sys.path.insert(0, "/opt/trn_rl_repo")

from contextlib import ExitStack

import numpy as np

import concourse.bass as bass
import concourse.tile as tile
from concourse import bacc, mybir

F32 = mybir.dt.float32
BF16 = mybir.dt.bfloat16
P = 128

B, S, IN, OUT = 8, 2048, 4096, 4096
T = B * S
TG, FG = 2, 4
T_C, F_C = T // TG, OUT // FG
N_CORES = 8
KT = IN // P     # 32 k-tiles
TT = T_C // P    # 64 token tiles
FB = F_C // 512  # 2 psum chains per token tile


def build_bass(repeat=1, xt_bufs=3, w_bufs=2, out_bufs=3, ps_bufs=8,
               w_chunks=8, x_dma="sync", w_dma="gpsimd", out_dma="scalar",
               evac_engine="vector", f_outer=True, split_out_dma=False,
               out_bf16=False):
    nc = bacc.Bacc("TRN2", target_bir_lowering=False, debug=False,
                   enable_asserts=False, num_devices=1)

    ODT = BF16 if out_bf16 else F32
    xt_ap = nc.dram_tensor("xt", [TT, P, IN], BF16, kind="ExternalInput").ap()
    wt_ap = nc.dram_tensor("wt", [P, KT, F_C], BF16,
                           kind="ExternalInput").ap()
    out_ap = nc.dram_tensor("out", [T_C, F_C], ODT, kind="ExternalOutput").ap()

    with tile.TileContext(nc) as tc:
        with ExitStack() as ctx:
            xtp = ctx.enter_context(tc.tile_pool(name="xtp", bufs=xt_bufs))
            wtp = ctx.enter_context(tc.tile_pool(name="wtp", bufs=w_bufs))
            outp = ctx.enter_context(tc.tile_pool(name="outp", bufs=out_bufs))
            psp = ctx.enter_context(
                tc.tile_pool(name="ps", bufs=ps_bufs, space="PSUM"))

            if repeat > 1:
                ctx.enter_context(tc.For_i(0, repeat, 1))

            # resident k-major weights, streamed in k-chunks so tile-0
            # matmuls can start as soon as the first chunk lands
            wt = wtp.tile([P, KT, F_C], BF16, tag="wt")
            kpc = KT // w_chunks
            for c in range(w_chunks):
                getattr(nc, w_dma).dma_start(
                    wt[:, c * kpc:(c + 1) * kpc, :],
                    wt_ap[:, c * kpc:(c + 1) * kpc, :])

            def load_x(t):
                xt = xtp.tile([P, IN], BF16, tag="xt", name="xt")
                getattr(nc, x_dma).dma_start(xt[:], xt_ap[t])
                return xt

            xts = {}
            for t in range(min(xt_bufs - 1, TT)):
                xts[t] = load_x(t)

            for t in range(TT):
                if t + xt_bufs - 1 < TT:
                    xts[t + xt_bufs - 1] = load_x(t + xt_bufs - 1)
                xt = xts.pop(t)

                pcs = [psp.tile([P, 512], F32, tag="ps", name=f"pc{f}")
                       for f in range(FB)]
                ot = outp.tile([P, F_C], ODT, tag="ot")
                if f_outer:
                    for f in range(FB):
                        for k in range(KT):
                            nc.tensor.matmul(
                                pcs[f][:], xt[:, k * P:(k + 1) * P],
                                wt[:, k, f * 512:(f + 1) * 512],
                                start=(k == 0), stop=(k == KT - 1))
                        getattr(nc, evac_engine).tensor_copy(
                            out=ot[:, f * 512:(f + 1) * 512], in_=pcs[f][:])
                        if split_out_dma:
                            getattr(nc, out_dma).dma_start(
                                out_ap[t * P:(t + 1) * P,
                                       f * 512:(f + 1) * 512],
                                ot[:, f * 512:(f + 1) * 512])
                else:
                    for k in range(KT):
                        for f in range(FB):
                            nc.tensor.matmul(
                                pcs[f][:], xt[:, k * P:(k + 1) * P],
                                wt[:, k, f * 512:(f + 1) * 512],
                                start=(k == 0), stop=(k == KT - 1))
                    for f in range(FB):
                        getattr(nc, evac_engine).tensor_copy(
                            out=ot[:, f * 512:(f + 1) * 512], in_=pcs[f][:])
                if not (f_outer and split_out_dma):
                    getattr(nc, out_dma).dma_start(
                        out_ap[t * P:(t + 1) * P, :], ot[:])

    nc.compile()
    return nc


class SpmdRunner:
    """Builds the sharded jitted callable once (mirrors
    concourse.bass2jax.run_bass_via_pjrt's multi-core branch) so repeated
    executions skip re-tracing and reuse the cached NEFF."""

    def __init__(self, nc, n_cores):
        import jax
        from jax.sharding import Mesh, PartitionSpec
        from jax.experimental.shard_map import shard_map
        from concourse.bass2jax import (
            _bass_exec_p, install_neuronx_cc_hook, partition_id_tensor)

        self.jax = jax
        self.PartitionSpec = PartitionSpec
        install_neuronx_cc_hook()
        assert nc.dbg_addr is None
        self.n_cores = n_cores
        partition_name = (
            nc.partition_id_tensor.name if nc.partition_id_tensor else None)
        in_names, out_names, out_avals, zero_outs = [], [], [], []
        for alloc in nc.m.functions[0].allocations:
            if not isinstance(alloc, mybir.MemoryLocationSet):
                continue
            name = alloc.memorylocations[0].name
            if alloc.kind == "ExternalInput":
                if name != partition_name:
                    in_names.append(name)
            elif alloc.kind == "ExternalOutput":
                shape = tuple(alloc.tensor_shape)
                dtype = mybir.dt.np(alloc.dtype)
                out_names.append(name)
                out_avals.append(jax.core.ShapedArray(shape, dtype))
                zero_outs.append(np.zeros(shape, dtype))
        n_params = len(in_names)
        n_outs = len(out_avals)
        full_in_names = list(in_names) + list(out_names)
        if partition_name is not None:
            full_in_names.append(partition_name)
        self.in_names = in_names
        self.out_names = out_names
        self.out_avals = out_avals
        self.zero_outs = zero_outs

        def _body(*args):
            operands = list(args)
            if partition_name is not None:
                operands.append(partition_id_tensor())
            outs = _bass_exec_p.bind(
                *operands,
                out_avals=tuple(out_avals),
                in_names=tuple(full_in_names),
                out_names=tuple(out_names),
                lowering_input_output_aliases=(),
                sim_require_finite=True,
                sim_require_nnan=True,
                nc=nc,
            )
            return tuple(outs)

        devices = jax.devices()[:n_cores]
        assert len(devices) == n_cores, (
            f"need {n_cores} cores, have {len(jax.devices())}")
        mesh = Mesh(np.asarray(devices), ("core",))
        in_specs = (PartitionSpec("core"),) * (n_params + n_outs)
        out_specs = (PartitionSpec("core"),) * n_outs
        donate = tuple(range(n_params, n_params + n_outs))
        self.sharded = jax.jit(
            shard_map(_body, mesh=mesh, in_specs=in_specs,
                      out_specs=out_specs, check_rep=False),
            donate_argnums=donate, keep_unused=True)
        self.mesh = mesh

    def prep_inputs(self, in_maps):
        from jax.sharding import NamedSharding

        sh = NamedSharding(self.mesh, self.PartitionSpec("core"))
        concat = [
            np.concatenate([np.asarray(in_maps[c][name])
                            for c in range(self.n_cores)], axis=0)
            for name in self.in_names
        ]
        out = [self.jax.device_put(a, sh) for a in concat]
        self.jax.block_until_ready(out)
        return out

    def zeros(self):
        import jax.numpy as jnp
        from jax.sharding import NamedSharding

        if not hasattr(self, "_zeros_fn"):
            shardings = tuple(
                NamedSharding(self.mesh, self.PartitionSpec("core"))
                for _ in self.zero_outs)
            shapes = [((self.n_cores * z.shape[0], *z.shape[1:]), z.dtype)
                      for z in self.zero_outs]
            self._zeros_fn = self.jax.jit(
                lambda: tuple(jnp.zeros(s, d) for s, d in shapes),
                out_shardings=shardings)
        out = self._zeros_fn()
        self.jax.block_until_ready(out)
        return list(out)

    def __call__(self, prepped_inputs, zeros=None):
        if zeros is None:
            zeros = self.zeros()
        out_arrs = self.sharded(*prepped_inputs, *zeros)
        self.jax.block_until_ready(out_arrs)
        return out_arrs

    def split_outputs(self, out_arrs):
        return [
            {name: np.asarray(out_arrs[i]).reshape(
                self.n_cores, *self.out_avals[i].shape)[c]
             for i, name in enumerate(self.out_names)}
            for c in range(self.n_cores)
        ]


_CACHE = {}

# HW A/B (interleaved rounds, R1->R9 slopes): out_bf16 ~-60us vs fp32 out
# (halves the 32 MB/core output DMA); ps6/k_outer/w_sync/deep_bufs/split_out
# were all within noise or worse.
BEST_KW = dict(out_bf16=True)


def _get_runner(repeat=1, **kw):
    kw = {**BEST_KW, **kw}
    key = (repeat, tuple(sorted(kw.items())))
    if key not in _CACHE:
        nc = build_bass(repeat=repeat, **kw)
        _CACHE[key] = SpmdRunner(nc, N_CORES)
    return _CACHE[key]


def _shard_inputs(input, base_weight, delta_signs, delta_scales):
    import ml_dtypes

    X = np.asarray(input, dtype=np.float32).reshape(T, IN)
    w = (np.asarray(base_weight, dtype=np.float32)
         + (2.0 * np.asarray(delta_signs, dtype=np.float32) - 1.0)
         * np.asarray(delta_scales, dtype=np.float32)[:, None])
    Xb = X.astype(ml_dtypes.bfloat16)
    wb = w.astype(ml_dtypes.bfloat16)

    xts = []
    for tg in range(TG):
        Xg = Xb[tg * T_C:(tg + 1) * T_C]
        # [t, tok, kt, p] -> [t, p, kt, tok]
        xt = np.ascontiguousarray(
            Xg.reshape(TT, P, KT, P).transpose(0, 3, 2, 1)).reshape(TT, P, IN)
        xts.append(xt)
    wts = []
    for fg in range(FG):
        Wg = wb[fg * F_C:(fg + 1) * F_C]
        # [f, kt, p] -> [p, kt, f]
        wts.append(np.ascontiguousarray(
            Wg.reshape(F_C, KT, P).transpose(2, 1, 0)))

    in_maps = []
    for c in range(N_CORES):
        tg, fg = divmod(c, FG)
        in_maps.append({"xt": xts[tg], "wt": wts[fg]})
    return in_maps


def kernel(input, base_weight, delta_signs, delta_scales):
    runner = _get_runner()
    in_maps = _shard_inputs(input, base_weight, delta_signs, delta_scales)
    prepped = runner.prep_inputs(in_maps)
    out_arrs = runner(prepped)
    res = runner.split_outputs(out_arrs)
    C = np.empty((T, OUT), np.float32)
    for c in range(N_CORES):
        tg, fg = divmod(c, FG)
        C[tg * T_C:(tg + 1) * T_C, fg * F_C:(fg + 1) * F_C] = (
            np.asarray(res[c]["out"], dtype=np.float32))
    return C.reshape(B, S, OUT)
